# revision 19
# baseline (speedup 1.0000x reference)
"""Bahdanau-style attention kernel for Trainium2, SPMD across 8 NeuronCores.

Math (per batch row b):
    dec_proj = decoder_state @ W_dec + b_transform            # [D]
    enc_proj = encoder_outputs[b] @ W_enc                     # [S, D]
    feats    = tanh(enc_proj + dec_proj)                      # [S, D]
    scores   = feats @ v_scorer                               # [S]
    probs    = softmax(where(mask, scores, -1e9))             # [S]

Distribution: data-parallel on batch (8 batches per core, weights
replicated), with mask-aware work packing: the mask is length-style, so
positions >= length contribute exactly 0 to the output. Work is split
into units of (batch, 128-column s-chunk) covering only unmasked
columns, and units are bin-packed across the 8 cores. Units are fused
into QUADS (4 x 128 cols) so the enc matmuls run at free-dim 512 (a
full PSUM bank). The Bass graph is parametrized only by P (quads per
core), so graphs are cached per P.

The device computes raw scores only (99.97% of the FLOPs); dec_proj
(0.02%) and the final masked softmax (tiny) run on host in f32/f64 --
this removes the whole device epilogue (mask matmul, exp, bf16 sums,
reciprocal, rescale) and shortens the tail.

Precision: the enc_proj contraction (E=1024) is split 768/256:
  - E-rows 0-767 run as fp8 e4m3 DoubleRow matmuls (2 rows/PE-cell,
    2x bf16 FLOP rate). enc is pre-scaled x16 and W_enc x64 to dodge
    fp8 subnormals; the 2^10 product scale is undone downstream.
  - E-rows 768-1023 stay bf16 (same x16/x64 scaling so both parts
    share one PSUM accumulation group).
  Measured end-to-end rel err ~1.84e-2 vs the f32 reference, under the
  2e-2 gate; fp8 on the full contraction measures ~2.2e-2 in numpy
  simulation (worse on HW) and is not viable.

Device-side formulation (per core, P quads = U units):
  - enc_projT[d, s] per quad in one [128, 512] PSUM bank: 3 DoubleRow
    matmuls (256 contraction rows each) + 2 bf16 matmuls (128 rows).
  - the PSUM drain is split across engines: DVE adds the per-unit
    dec_proj bias (x1024 pre-scaled) for quarters 0-1 then ScalarE
    applies a no-bias tanh to them, and ScalarE handles quarters 2-3
    directly as biased tanh ACTs; output ft [128, 4, 128] bf16.
  - Scores: ONE matmul per (quad, d-tile): lhsT = a [128, U] one-hot-
    quad selector (stride trick: 4 consecutive v-copies sliced per
    quad; PE out base partition must be 0/32/64 so partition-sliced
    outs are not an option) x rhs = ft [128, 512] -> accumulates rows
    4p..4p+3 of scU [U, 512], zeros elsewhere. Row u only has valid
    scores in its own 128-col chunk (h = u%4); the junk in other
    chunks is discarded on host. This quarters the score-matmul
    instruction count vs one-hot per-unit matmuls (same streamed
    cycles, far less per-instruction overhead).
  - Score matmuls are delayed by one quad so the in-order PE never
    stalls on the DVE+ACT drain pipeline.

Startup: the first DR matmul only needs 128KB of weights + 128KB of
rhs, so wenc8 / quad-0 enc8 / quad-0 encb are DMA'd in pass-sized
pieces, spread over four rings (scalar/sync/gpsimd/vector), and quad 0
runs its matmuls pr-major (pass-major) with paired bf16 closes so
compute starts as soon as the first pieces land. Steady-state quads
stay t-major (drains spread evenly) with whole-tile DMAs on the
sync (fp8) and gpsimd (bf16) rings.
"""

import math

import numpy as np
import ml_dtypes

B, S, E, D = 64, 1024, 1024, 512
N_CORES = 8
BPC = B // N_CORES  # batches per core
ND = D // 128  # 4 d-tiles
CH = 128  # columns per work unit
GRP = 4  # units fused into one matmul tile
FD = GRP * CH  # matmul free dim (512, a full PSUM bank)
EF = 768  # contraction rows done in fp8 (0..767)
NPASS = EF // 256  # 3 DoubleRow passes (256 rows each)
NEB = (E - EF) // 128  # 2 bf16 e-tiles (rows 768..1023)
ENC_SCALE = 16.0
W_SCALE = 64.0
PSUM_SCALE = 1.0 / (ENC_SCALE * W_SCALE)
NEG_INF = -1000000000.0

_cache = {}


def _build(P):
    """Build + compile the SPMD graph for P unit-quads per core."""
    from contextlib import ExitStack

    import concourse.bass as bass
    import concourse.tile as tile
    from concourse import bacc, mybir

    f32 = mybir.dt.float32
    bf16 = mybir.dt.bfloat16
    f8 = mybir.dt.float8e4
    AF = mybir.ActivationFunctionType
    DR = mybir.MatmulPerfMode.DoubleRow

    U = GRP * P

    nc = bacc.Bacc(
        "TRN2", target_bir_lowering=False, debug=False, num_devices=N_CORES
    )

    # pr-major weight layouts so quad-0's pass-piece DMAs are contiguous
    enc8 = nc.dram_tensor("enc8", [P, 128, NPASS, 2, FD], f8, kind="ExternalInput").ap()
    encb = nc.dram_tensor("encb", [P, 128, NEB, FD], bf16, kind="ExternalInput").ap()
    wenc8 = nc.dram_tensor("wenc8", [128, NPASS, 2, ND, 128], f8, kind="ExternalInput").ap()
    wencb = nc.dram_tensor("wencb", [128, NEB, ND, 128], bf16, kind="ExternalInput").ap()
    dproj = nc.dram_tensor("dproj", [128, ND, U], f32, kind="ExternalInput").ap()
    # only the 4 nonzero v-columns come over DMA; the [128, ND, 2U] one-hot-
    # quad selector (stride trick: slice [:, t, U-4p : 2U-4p] is a [128, U]
    # matrix whose only nonzero columns are 4p..4p+3, each = v tile t) is
    # assembled on device by the otherwise-idle DVE to save startup HBM BW
    vsm = nc.dram_tensor("vsm", [128, ND, GRP], bf16, kind="ExternalInput").ap()
    out = nc.dram_tensor("out", [U, FD], f32, kind="ExternalOutput").ap()

    with tile.TileContext(nc) as tc:
        with ExitStack() as ctx:
            const = ctx.enter_context(tc.tile_pool(name="const", bufs=1))
            e8_pool = ctx.enter_context(tc.tile_pool(name="e8p", bufs=5))
            eb_pool = ctx.enter_context(tc.tile_pool(name="ebp", bufs=5))
            fpool = ctx.enter_context(tc.tile_pool(name="feats", bufs=10))
            fpre_pool = ctx.enter_context(tc.tile_pool(name="fpre", bufs=4))

            # PE warmup: the memset is the very first gpsimd instruction so
            # the dependency-free dummy matmuls can start ASAP, fill the
            # startup DMA wait, and trip the HAM clock-gate to 2.4 GHz.
            # Small tile -> short memset -> warmups start right after the
            # fixed ~6us engine preamble.
            warm_sb = const.tile([128, 384], bf16)
            nc.gpsimd.memset(warm_sb[:], 0.0)

            # Startup DMA schedule. Aggregate DMA BW is ~230 B/ns across the
            # three queues, and dependency tracking is per-TILE, so quad-0's
            # data is split into piece tiles and every transfer is queued in
            # strict PE-need order; quad 1's fp8 tile is split across all
            # three rings so no single ring gates it.
            #   scalar: wenc8 pass tiles, quad-0 dproj, e81-pr2, e82-pr2,
            #           dproj rest, v columns
            #   sync:   quad-0 enc8 pass tiles, e81-pr0, e82-pr01, quads 3+
            #   gpsimd: wencb/encb interleaved per e-tile, e81-pr1, eb1, eb2
            # NOTE: untagged same-size tiles in a pool rotate through one
            # slot ring -- every piece tile needs its own tag (bufs=1)
            wenc8_prs = []
            for pr in range(NPASS):
                t8 = const.tile([128, 2, ND, 128], f8, tag=f"w8p{pr}", name="w8p")
                nc.scalar.dma_start(t8[:], wenc8[:, pr])
                wenc8_prs.append(t8)
            e80_prs = []
            for pr in range(NPASS):
                tp = e8_pool.tile(
                    [128, 2, FD], f8, tag=f"e80p{pr}", name="e80p", bufs=1
                )
                nc.sync.dma_start(tp[:], enc8[0][:, pr])
                e80_prs.append(tp)
            dproj_sb = const.tile([128, ND, U], f32)
            dproj0_sb = const.tile([128, ND, GRP], f32)
            nc.scalar.dma_start(dproj0_sb[:], dproj[:, :, 0:GRP])
            wencb_sb = const.tile([128, NEB, ND, 128], bf16)
            eb0_es = []
            for e in range(NEB):
                nc.gpsimd.dma_start(wencb_sb[:, e], wencb[:, e])
                te_ = eb_pool.tile(
                    [128, FD], bf16, tag=f"eb0e{e}", name="eb0e", bufs=1
                )
                nc.gpsimd.dma_start(te_[:], encb[0][:, e])
                eb0_es.append(te_)
            # quad 1: fp8 split 3 ways; quad 2: sync pr01 + scalar pr2
            e8s, ebs = [None], [None]
            npre = min(3, P)
            e81t = e8_pool.tile([128, NPASS, 2, FD], f8, tag="e8", name="e8")
            nc.sync.dma_start(e81t[:, 0:1], enc8[1][:, 0:1])
            nc.gpsimd.dma_start(e81t[:, 1:2], enc8[1][:, 1:2])
            nc.scalar.dma_start(e81t[:, 2:NPASS], enc8[1][:, 2:NPASS])
            e8s.append(e81t)
            eb1t = eb_pool.tile([128, NEB, FD], bf16, tag="eb", name="eb")
            nc.gpsimd.dma_start(eb1t[:], encb[1])
            ebs.append(eb1t)
            if P > 2:
                e82t = e8_pool.tile([128, NPASS, 2, FD], f8, tag="e8", name="e8")
                nc.sync.dma_start(e82t[:, 0:2], enc8[2][:, 0:2])
                nc.scalar.dma_start(e82t[:, 2:NPASS], enc8[2][:, 2:NPASS])
                e8s.append(e82t)
                eb2t = eb_pool.tile([128, NEB, FD], bf16, tag="eb", name="eb")
                nc.gpsimd.dma_start(eb2t[:], encb[2])
                ebs.append(eb2t)
            nc.scalar.dma_start(dproj_sb[:, :, GRP:U], dproj[:, :, GRP:U])
            vsm_sb = const.tile([128, ND, GRP], bf16, tag="vsm", name="vsm")
            nc.scalar.dma_start(vsm_sb[:], vsm)
            # DVE assembles the one-hot-quad selector from the v columns
            vQ_sb = const.tile([128, ND, 2 * U], bf16)
            nc.vector.memset(vQ_sb[:], 0.0)
            nc.vector.tensor_copy(vQ_sb[:, :, U : U + GRP], vsm_sb[:])

            with tc.tile_pool(name="warmp", bufs=1, space="PSUM") as wpool:
                wps = wpool.tile([128, 256], f32, name="wps")
                for _ in range(12):
                    nc.tensor.matmul(
                        wps[:],
                        lhsT=warm_sb[:, 0:128],
                        rhs=warm_sb[:, 128:384],
                        start=True,
                        stop=True,
                        skip_group_check=True,
                    )

            spsum = ctx.enter_context(tc.tile_pool(name="spsum", bufs=1, space="PSUM"))
            scU = spsum.tile([U, FD], f32, name="scU")
            n_sc_mms = P * ND
            state = {"count": 0}
            pending = []  # delayed score MMs: (p, t, ft)

            def flush_pending():
                for (p, t, ft) in pending:
                    state["count"] += 1
                    nc.tensor.matmul(
                        scU[:],
                        lhsT=vQ_sb[:, t, U - GRP * p : 2 * U - GRP * p],
                        rhs=ft[:],
                        start=(state["count"] == 1),
                        stop=(state["count"] == n_sc_mms),
                        skip_group_check=True,
                    )
                pending.clear()

            def drain(ps, p, t):
                """PSUM [128, 512] -> ft bf16 via DVE scale+bias + ScalarE tanh."""
                NV = 2
                dp = dproj0_sb if p == 0 else dproj_sb
                u0 = 0 if p == 0 else GRP * p
                fpre = fpre_pool.tile([128, NV, CH], bf16, tag="fp", name="fp")
                for h in range(NV):
                    # fpre = ps * 2^-10 + dproj[u] in one DVE op (scaled
                    # imm first, then per-partition bias add)
                    nc.vector.tensor_scalar(
                        fpre[:, h, :],
                        ps[:, h, :],
                        PSUM_SCALE,
                        dp[:, t, u0 + h : u0 + h + 1],
                        mybir.AluOpType.mult,
                        mybir.AluOpType.add,
                    )
                ft = fpool.tile([128, GRP, CH], bf16, tag="ft", name="ft")
                for h in range(NV, GRP):
                    nc.scalar.activation(
                        ft[:, h, :],
                        ps[:, h, :],
                        func=AF.Tanh,
                        bias=dp[:, t, u0 + h : u0 + h + 1],
                        scale=PSUM_SCALE,
                    )
                nc.scalar.activation(
                    ft[:, 0:NV, :], fpre[:], func=AF.Tanh, scale=1.0
                )
                pending.append((p, t, ft))

            # --- main loop over quads ---
            with tc.tile_pool(name="mpsum", bufs=7, space="PSUM") as mpsum:
                for p in range(P):
                    if p == 0:
                        e8t, ebt = None, None
                    elif p < npre:
                        e8t, ebt = e8s[p], ebs[p]
                    else:
                        e8t = e8_pool.tile([128, NPASS, 2, FD], f8, tag="e8", name="e8")
                        nc.sync.dma_start(e8t[:], enc8[p])
                        ebt = eb_pool.tile([128, NEB, FD], bf16, tag="eb", name="eb")
                        # bf16 tile rides the gpsimd ring: one ring tops out
                        # at ~102 B/ns; splitting fp8/bf16 across two rings
                        # keeps DMA under the PE floor
                        nc.gpsimd.dma_start(ebt[:], encb[p])
                    prev = list(pending)
                    pending.clear()
                    pss = [
                        mpsum.tile([128, GRP, CH], f32, tag="mp", name="mp")
                        for _ in range(ND)
                    ]
                    if p == 0:
                        # quad 0: pass-major so compute starts when the first
                        # 128KB weight + rhs piece tiles land; bf16 closes
                        # paired per d-tile so drains start at matmul 14
                        for pr in range(NPASS):
                            for t in range(ND):
                                nc.tensor.matmul(
                                    pss[t][:],
                                    lhsT=wenc8_prs[pr][:, :, t, :],
                                    rhs=e80_prs[pr][:],
                                    start=(pr == 0),
                                    stop=False,
                                    perf_mode=DR,
                                )
                        for t in range(ND):
                            for e in range(NEB):
                                nc.tensor.matmul(
                                    pss[t][:],
                                    lhsT=wencb_sb[:, e, t, :],
                                    rhs=eb0_es[e][:],
                                    start=False,
                                    stop=(e == NEB - 1),
                                )
                            drain(pss[t], p, t)
                    else:
                        # steady state: t-major, fp8 first (their data leads
                        # the DMA stream), bf16 closes each group, drain per
                        # d-tile so ScalarE/DVE work is spread evenly
                        for t in range(ND):
                            for pr in range(NPASS):
                                nc.tensor.matmul(
                                    pss[t][:],
                                    lhsT=wenc8_prs[pr][:, :, t, :],
                                    rhs=e8t[:, pr, :, :],
                                    start=(pr == 0),
                                    stop=False,
                                    perf_mode=DR,
                                )
                        for t in range(ND):
                            for e in range(NEB):
                                nc.tensor.matmul(
                                    pss[t][:],
                                    lhsT=wencb_sb[:, e, t, :],
                                    rhs=ebt[:, e, :],
                                    start=False,
                                    stop=(e == NEB - 1),
                                )
                            drain(pss[t], p, t)
                    # emit previous quad's score MMs now (their tanh inputs
                    # are ready, so PE doesn't stall on ACT)
                    this_pair = pending[:]
                    pending.clear()
                    pending.extend(prev)
                    flush_pending()
                    pending.extend(this_pair)
                flush_pending()

            # --- tail: raw scores PSUM -> SBUF -> HBM (softmax on host).
            # out DMAs ride sync + scalar; the gpsimd queue drains slowly
            # at teardown so keep it clean at the end ---
            with tc.tile_pool(name="epi", bufs=1) as epool:
                scores_sb = epool.tile([U, FD], f32, name="scores_sb")
                H = FD // 2
                nc.vector.tensor_copy(scores_sb[:, 0:H], scU[:, 0:H])
                nc.sync.dma_start(out[:, 0:H], scores_sb[:, 0:H])
                nc.scalar.activation(
                    scores_sb[:, H:FD], scU[:, H:FD], func=AF.Copy
                )
                nc.scalar.dma_start(out[:, H:FD], scores_sb[:, H:FD])

    nc.compile()
    return nc


def _assign(lengths):
    """Bin-pack batches (weight = #units) onto 8 cores, 8 batches each.

    Returns (per_core_batches, U) where per_core_batches[i] is a list of
    8 global batch indices and U is the max unit count.
    """
    w = [max(1, math.ceil(l / CH)) for l in lengths]
    order = sorted(range(B), key=lambda b: -w[b])
    loads = [0] * N_CORES
    members = [[] for _ in range(N_CORES)]
    for b in order:
        cands = [i for i in range(N_CORES) if len(members[i]) < BPC]
        i = min(cands, key=lambda i: (loads[i], len(members[i])))
        members[i].append(b)
        loads[i] += w[b]
    U = max(loads)
    return members, U


def _prep_inputs(decoder_state, encoder_outputs, input_mask, W_transform,
                 b_transform, v_scorer, members, P):
    bf16 = ml_dtypes.bfloat16
    f8 = ml_dtypes.float8_e4m3
    U = GRP * P
    W_dec = W_transform[:D]
    W_enc = W_transform[D:]

    # fp8 half of W (E-rows 0..EF), x64 scale, pr-major for DoubleRow:
    # contraction row e = pr*256 + ko*128 + partition
    w8 = (W_enc[:EF] * W_SCALE).astype(f8)  # [EF, D]
    wenc8_h = np.ascontiguousarray(
        w8.reshape(NPASS, 2, 128, ND, 128).transpose(2, 0, 1, 3, 4)
    )  # [128, NPASS, 2, ND, 128]
    wb = (W_enc[EF:] * W_SCALE).astype(bf16)  # [E-EF, D]
    wencb_h = np.ascontiguousarray(
        wb.reshape(NEB, 128, ND, 128).transpose(1, 0, 2, 3)
    )  # [128, NEB, ND, 128]
    v_tiles = v_scorer.astype(np.float32).reshape(ND, 128).T.astype(bf16)  # [128, ND]
    vsm_h = np.ascontiguousarray(
        np.repeat(v_tiles[:, :, None], GRP, axis=2)
    )  # [128, ND, GRP]

    # dec_proj on host (0.02% of the FLOPs): [B, D]
    dec_proj = (decoder_state.astype(np.float32) @ W_dec.astype(np.float32)
                + b_transform.astype(np.float32))

    lengths = input_mask.sum(axis=1).astype(int)
    enc8_full = (encoder_outputs[:, :, :EF] * ENC_SCALE).astype(f8)  # [B, S, EF]
    encb_full = (encoder_outputs[:, :, EF:] * ENC_SCALE).astype(bf16)  # [B, S, E-EF]

    in_maps = []
    unit_maps = []  # per core: list of (global_batch, chunk) per unit
    for core in range(N_CORES):
        batches = members[core]
        units = []
        for gb in batches:
            for c in range(max(1, math.ceil(lengths[gb] / CH))):
                units.append((gb, c))
        assert len(units) <= U
        enc8_h = np.zeros((P, 128, NPASS, 2, FD), dtype=f8)
        encb_h = np.zeros((P, 128, NEB, FD), dtype=bf16)
        dproj_h = np.zeros((128, ND, U), dtype=np.float32)
        for u, (gb, c) in enumerate(units):
            s0 = c * CH
            s1 = min(s0 + CH, int(lengths[gb]))
            n = s1 - s0
            p, h = divmod(u, GRP)
            col = h * CH
            # [n, EF] -> [EF, n] -> [NPASS, 2, 128, n] -> [128, NPASS, 2, n]
            blk8 = enc8_full[gb, s0:s1].T.reshape(NPASS, 2, 128, n)
            enc8_h[p, :, :, :, col : col + n] = blk8.transpose(2, 0, 1, 3)
            blkb = encb_full[gb, s0:s1].T.reshape(NEB, 128, n)
            encb_h[p, :, :, col : col + n] = blkb.transpose(1, 0, 2)
            dproj_h[:, :, u] = dec_proj[gb].reshape(ND, 128).T
        in_maps.append(
            {
                "enc8": enc8_h,
                "encb": encb_h,
                "wenc8": wenc8_h,
                "wencb": wencb_h,
                "dproj": dproj_h,
                "vsm": vsm_h,
            }
        )
        unit_maps.append(units)
    return in_maps, unit_maps


def kernel(decoder_state, encoder_outputs, input_mask, W_transform, b_transform,
           v_scorer, _trace=False):
    from concourse.bass_utils import run_bass_kernel_spmd

    decoder_state = np.asarray(decoder_state)
    encoder_outputs = np.asarray(encoder_outputs)
    input_mask = np.asarray(input_mask)
    W_transform = np.asarray(W_transform)
    b_transform = np.asarray(b_transform)
    v_scorer = np.asarray(v_scorer)

    lengths = input_mask.sum(axis=1).astype(int)
    members, U = _assign(lengths)
    P = (U + GRP - 1) // GRP

    key = ("nc", P)
    if key not in _cache:
        _cache[key] = _build(P)
    nc = _cache[key]

    in_maps, unit_maps = _prep_inputs(
        decoder_state, encoder_outputs, input_mask, W_transform, b_transform,
        v_scorer, members, P
    )
    res = run_bass_kernel_spmd(nc, in_maps, core_ids=list(range(N_CORES)), trace=_trace)

    # gather raw scores, then masked softmax on host (f64 exp, f32 out)
    scores = np.full((B, S), NEG_INF, dtype=np.float32)
    for core in range(N_CORES):
        o = res.results[core]["out"]  # [U, FD]
        for u, (gb, c) in enumerate(unit_maps[core]):
            h = u % GRP
            n = min(CH, int(lengths[gb]) - c * CH)
            scores[gb, c * CH : c * CH + n] = o[u, h * CH : h * CH + n]
    smax = scores.max(axis=1, keepdims=True)
    ex = np.exp((scores - smax).astype(np.float64))
    out_full = (ex / ex.sum(axis=1, keepdims=True)).astype(np.float32)
    if _trace:
        _cache["last_result"] = res
    return out_full


# revision 24
# speedup vs baseline: 1.0602x; 1.0602x over previous
"""Bahdanau-style attention kernel for Trainium2, SPMD across 8 NeuronCores.

Math (per batch row b):
    dec_proj = decoder_state @ W_dec + b_transform            # [D]
    enc_proj = encoder_outputs[b] @ W_enc                     # [S, D]
    feats    = tanh(enc_proj + dec_proj)                      # [S, D]
    scores   = feats @ v_scorer                               # [S]
    probs    = softmax(where(mask, scores, -1e9))             # [S]

Distribution: data-parallel on batch (8 batches per core, weights
replicated), with mask-aware work packing: the mask is length-style, so
positions >= length contribute exactly 0 to the output. Work is split
into units of (batch, 128-column s-chunk) covering only unmasked
columns, and units are bin-packed across the 8 cores. Units are fused
into QUADS (4 x 128 cols) so the enc matmuls run at free-dim 512 (a
full PSUM bank). The Bass graph is parametrized only by P (quads per
core), so graphs are cached per P.

The device computes raw scores only (99.97% of the FLOPs); dec_proj
(0.02%) and the final masked softmax (tiny) run on host in f32/f64 --
this removes the whole device epilogue (mask matmul, exp, bf16 sums,
reciprocal, rescale) and shortens the tail.

Precision: the enc_proj contraction (E=1024) is split 768/256:
  - E-rows 0-767 run as fp8 e4m3 DoubleRow matmuls (2 rows/PE-cell,
    2x bf16 FLOP rate). enc is pre-scaled x16 and W_enc x64 to dodge
    fp8 subnormals; the 2^10 product scale is undone downstream.
  - E-rows 768-1023 stay bf16 (same x16/x64 scaling so both parts
    share one PSUM accumulation group).
  Measured end-to-end rel err ~1.84e-2 vs the f32 reference, under the
  2e-2 gate; fp8 on the full contraction measures ~2.2e-2 in numpy
  simulation (worse on HW) and is not viable.

Device-side formulation (per core, P quads = U units):
  - enc_projT[d, s] per quad in one [128, 512] PSUM bank: 3 DoubleRow
    matmuls (256 contraction rows each) + 2 bf16 matmuls (128 rows).
  - the PSUM drain is split across engines: DVE adds the per-unit
    dec_proj bias (x1024 pre-scaled) for quarters 0-1 then ScalarE
    applies a no-bias tanh to them, and ScalarE handles quarters 2-3
    directly as biased tanh ACTs; output ft [128, 4, 128] bf16.
  - Scores: ONE matmul per (quad, d-tile): lhsT = a [128, U] one-hot-
    quad selector (stride trick: 4 consecutive v-copies sliced per
    quad; PE out base partition must be 0/32/64 so partition-sliced
    outs are not an option) x rhs = ft [128, 512] -> accumulates rows
    4p..4p+3 of scU [U, 512], zeros elsewhere. Row u only has valid
    scores in its own 128-col chunk (h = u%4); the junk in other
    chunks is discarded on host. This quarters the score-matmul
    instruction count vs one-hot per-unit matmuls (same streamed
    cycles, far less per-instruction overhead).
  - Score matmuls are delayed by one quad so the in-order PE never
    stalls on the DVE+ACT drain pipeline.

Startup: the first DR matmul only needs 128KB of weights + 128KB of
rhs, so wenc8 / quad-0 enc8 / quad-0 encb are DMA'd in pass-sized
pieces, spread over four rings (scalar/sync/gpsimd/vector), and quad 0
runs its matmuls pr-major (pass-major) with paired bf16 closes so
compute starts as soon as the first pieces land. Steady-state quads
stay t-major (drains spread evenly) with whole-tile DMAs on the
sync (fp8) and gpsimd (bf16) rings.
"""

import math

import numpy as np
import ml_dtypes

B, S, E, D = 64, 1024, 1024, 512
N_CORES = 8
BPC = B // N_CORES  # batches per core
ND = D // 128  # 4 d-tiles
CH = 128  # columns per work unit
GRP = 4  # units fused into one matmul tile
FD = GRP * CH  # matmul free dim (512, a full PSUM bank)
EF = 768  # contraction rows done in fp8 (0..767)
NPASS = EF // 256  # 3 DoubleRow passes (256 rows each)
NEB = (E - EF) // 128  # 2 bf16 e-tiles (rows 768..1023)
ENC_SCALE = 16.0
W_SCALE = 64.0
PSUM_SCALE = 1.0 / (ENC_SCALE * W_SCALE)
NEG_INF = -1000000000.0

_cache = {}


def _build(P):
    """Build + compile the SPMD graph for P unit-quads per core."""
    from contextlib import ExitStack

    import concourse.bass as bass
    import concourse.tile as tile
    from concourse import bacc, mybir

    f32 = mybir.dt.float32
    bf16 = mybir.dt.bfloat16
    f8 = mybir.dt.float8e4
    AF = mybir.ActivationFunctionType
    DR = mybir.MatmulPerfMode.DoubleRow

    U = GRP * P

    nc = bacc.Bacc(
        "TRN2", target_bir_lowering=False, debug=False, num_devices=N_CORES
    )

    # pr-major weight layouts so quad-0's pass-piece DMAs are contiguous
    enc8 = nc.dram_tensor("enc8", [P, 128, NPASS, 2, FD], f8, kind="ExternalInput").ap()
    encb = nc.dram_tensor("encb", [P, 128, NEB, FD], bf16, kind="ExternalInput").ap()
    wenc8 = nc.dram_tensor("wenc8", [128, NPASS, 2, ND, 128], f8, kind="ExternalInput").ap()
    wencb = nc.dram_tensor("wencb", [128, NEB, ND, 128], bf16, kind="ExternalInput").ap()
    # quad-0's dproj slice is its own (contiguous) tensor so its DMA is a
    # handful of big packets, not 512 16-byte strided ones
    dproj0 = nc.dram_tensor("dproj0", [128, ND, GRP], f32, kind="ExternalInput").ap()
    dprojr = nc.dram_tensor("dprojr", [128, ND, U - GRP], f32, kind="ExternalInput").ap()
    # only the 4 nonzero v-columns come over DMA; the [128, ND, 2U] one-hot-
    # quad selector (stride trick: slice [:, t, U-4p : 2U-4p] is a [128, U]
    # matrix whose only nonzero columns are 4p..4p+3, each = v tile t) is
    # assembled on device by the otherwise-idle DVE to save startup HBM BW
    vsm = nc.dram_tensor("vsm", [128, ND, GRP], bf16, kind="ExternalInput").ap()
    out = nc.dram_tensor("out", [U, FD], f32, kind="ExternalOutput").ap()

    with tile.TileContext(nc) as tc:
        with ExitStack() as ctx:
            const = ctx.enter_context(tc.tile_pool(name="const", bufs=1))
            e8_pool = ctx.enter_context(tc.tile_pool(name="e8p", bufs=5))
            eb_pool = ctx.enter_context(tc.tile_pool(name="ebp", bufs=5))
            fpool = ctx.enter_context(tc.tile_pool(name="feats", bufs=10))
            fpre_pool = ctx.enter_context(tc.tile_pool(name="fpre", bufs=4))

            # PE warmup: the memset is the very first gpsimd instruction so
            # the dependency-free dummy matmuls can start ASAP, fill the
            # startup DMA wait, and trip the HAM clock-gate to 2.4 GHz.
            # Small tile -> short memset -> warmups start right after the
            # fixed ~6us engine preamble.
            warm_sb = const.tile([128, 384], bf16)
            nc.gpsimd.memset(warm_sb[:], 0.0)

            # Startup DMA schedule. Dependency tracking is per-TILE, so
            # quad-0's data is split into piece TILES so the first matmuls
            # unblock after 128KB. Ring assignment stays simple (mixing enc
            # pieces onto the scalar queue measures ~3x slower than keeping
            # it weights-only):
            #   scalar: wenc8 pass tiles, quad-0 dproj, dproj rest, v cols
            #   sync:   quad-0 enc8 pass tiles, then whole fp8 quads
            #   gpsimd: wencb/encb quad-0 pieces interleaved, whole bf16 quads
            # NOTE: untagged same-size tiles in a pool rotate through one
            # slot ring -- every piece tile needs its own tag (bufs=1)
            wenc8_prs = []
            for pr in range(NPASS):
                t8 = const.tile([128, 2, ND, 128], f8, tag=f"w8p{pr}", name="w8p")
                nc.scalar.dma_start(t8[:], wenc8[:, pr])
                wenc8_prs.append(t8)
            e80_prs = []
            for pr in range(NPASS):
                tp = e8_pool.tile(
                    [128, 2, FD], f8, tag=f"e80p{pr}", name="e80p", bufs=1
                )
                nc.sync.dma_start(tp[:], enc8[0][:, pr])
                e80_prs.append(tp)
            dproj0_sb = const.tile([128, ND, GRP], f32)
            nc.scalar.dma_start(dproj0_sb[:], dproj0)
            wencb_sb = const.tile([128, NEB, ND, 128], bf16)
            eb0_es = []
            for e in range(NEB):
                nc.gpsimd.dma_start(wencb_sb[:, e], wencb[:, e])
                te_ = eb_pool.tile(
                    [128, FD], bf16, tag=f"eb0e{e}", name="eb0e", bufs=1
                )
                nc.gpsimd.dma_start(te_[:], encb[0][:, e])
                eb0_es.append(te_)
            e8s, ebs = [None], [None]
            npre = min(3, P)
            for p in range(1, npre):
                e8t = e8_pool.tile([128, NPASS, 2, FD], f8, tag="e8", name="e8")
                nc.sync.dma_start(e8t[:], enc8[p])
                e8s.append(e8t)
                ebt = eb_pool.tile([128, NEB, FD], bf16, tag="eb", name="eb")
                nc.gpsimd.dma_start(ebt[:], encb[p])
                ebs.append(ebt)
            dprojr_sb = const.tile([128, ND, U - GRP], f32)
            nc.scalar.dma_start(dprojr_sb[:], dprojr)
            vsm_sb = const.tile([128, ND, GRP], bf16, tag="vsm", name="vsm")
            nc.scalar.dma_start(vsm_sb[:], vsm)
            # DVE assembles the one-hot-quad selector from the v columns
            vQ_sb = const.tile([128, ND, 2 * U], bf16)
            nc.vector.memset(vQ_sb[:], 0.0)
            nc.vector.tensor_copy(vQ_sb[:, :, U : U + GRP], vsm_sb[:])

            with tc.tile_pool(name="warmp", bufs=1, space="PSUM") as wpool:
                wps = wpool.tile([128, 256], f32, name="wps")
                for _ in range(12):
                    nc.tensor.matmul(
                        wps[:],
                        lhsT=warm_sb[:, 0:128],
                        rhs=warm_sb[:, 128:384],
                        start=True,
                        stop=True,
                        skip_group_check=True,
                    )

            spsum = ctx.enter_context(tc.tile_pool(name="spsum", bufs=1, space="PSUM"))
            scU = spsum.tile([U, FD], f32, name="scU")
            n_sc_mms = P * ND
            state = {"count": 0}
            pending = []  # delayed score MMs: (p, t, ft)

            def flush_pending():
                for (p, t, ft) in pending:
                    state["count"] += 1
                    nc.tensor.matmul(
                        scU[:],
                        lhsT=vQ_sb[:, t, U - GRP * p : 2 * U - GRP * p],
                        rhs=ft[:],
                        start=(state["count"] == 1),
                        stop=(state["count"] == n_sc_mms),
                        skip_group_check=True,
                    )
                pending.clear()

            def drain(ps, p, t):
                """PSUM [128, 512] -> ft bf16 via DVE scale+bias + ScalarE tanh."""
                NV = 2
                dp = dproj0_sb if p == 0 else dprojr_sb
                u0 = 0 if p == 0 else GRP * (p - 1)
                fpre = fpre_pool.tile([128, NV, CH], bf16, tag="fp", name="fp")
                for h in range(NV):
                    # fpre = ps * 2^-10 + dproj[u] in one DVE op (scaled
                    # imm first, then per-partition bias add)
                    nc.vector.tensor_scalar(
                        fpre[:, h, :],
                        ps[:, h, :],
                        PSUM_SCALE,
                        dp[:, t, u0 + h : u0 + h + 1],
                        mybir.AluOpType.mult,
                        mybir.AluOpType.add,
                    )
                ft = fpool.tile([128, GRP, CH], bf16, tag="ft", name="ft")
                for h in range(NV, GRP):
                    nc.scalar.activation(
                        ft[:, h, :],
                        ps[:, h, :],
                        func=AF.Tanh,
                        bias=dp[:, t, u0 + h : u0 + h + 1],
                        scale=PSUM_SCALE,
                    )
                nc.scalar.activation(
                    ft[:, 0:NV, :], fpre[:], func=AF.Tanh, scale=1.0
                )
                pending.append((p, t, ft))

            # --- main loop over quads ---
            with tc.tile_pool(name="mpsum", bufs=7, space="PSUM") as mpsum:
                for p in range(P):
                    if p == 0:
                        e8t, ebt = None, None
                    elif p < npre:
                        e8t, ebt = e8s[p], ebs[p]
                    else:
                        e8t = e8_pool.tile([128, NPASS, 2, FD], f8, tag="e8", name="e8")
                        nc.sync.dma_start(e8t[:], enc8[p])
                        ebt = eb_pool.tile([128, NEB, FD], bf16, tag="eb", name="eb")
                        # bf16 tile rides the gpsimd ring: one ring tops out
                        # at ~102 B/ns; splitting fp8/bf16 across two rings
                        # keeps DMA under the PE floor
                        nc.gpsimd.dma_start(ebt[:], encb[p])
                    prev = list(pending)
                    pending.clear()
                    pss = [
                        mpsum.tile([128, GRP, CH], f32, tag="mp", name="mp")
                        for _ in range(ND)
                    ]
                    if p == 0:
                        # quad 0: pass-major so compute starts when the first
                        # 128KB weight + rhs piece tiles land; bf16 closes
                        # paired per d-tile so drains start at matmul 14
                        for pr in range(NPASS):
                            for t in range(ND):
                                nc.tensor.matmul(
                                    pss[t][:],
                                    lhsT=wenc8_prs[pr][:, :, t, :],
                                    rhs=e80_prs[pr][:],
                                    start=(pr == 0),
                                    stop=False,
                                    perf_mode=DR,
                                )
                        for t in range(ND):
                            for e in range(NEB):
                                nc.tensor.matmul(
                                    pss[t][:],
                                    lhsT=wencb_sb[:, e, t, :],
                                    rhs=eb0_es[e][:],
                                    start=False,
                                    stop=(e == NEB - 1),
                                )
                            drain(pss[t], p, t)
                    else:
                        # steady state: t-major, fp8 first (their data leads
                        # the DMA stream), bf16 closes each group, drain per
                        # d-tile so ScalarE/DVE work is spread evenly
                        for t in range(ND):
                            for pr in range(NPASS):
                                nc.tensor.matmul(
                                    pss[t][:],
                                    lhsT=wenc8_prs[pr][:, :, t, :],
                                    rhs=e8t[:, pr, :, :],
                                    start=(pr == 0),
                                    stop=False,
                                    perf_mode=DR,
                                )
                        for t in range(ND):
                            for e in range(NEB):
                                nc.tensor.matmul(
                                    pss[t][:],
                                    lhsT=wencb_sb[:, e, t, :],
                                    rhs=ebt[:, e, :],
                                    start=False,
                                    stop=(e == NEB - 1),
                                )
                            drain(pss[t], p, t)
                    # emit previous quad's score MMs now (their tanh inputs
                    # are ready, so PE doesn't stall on ACT)
                    this_pair = pending[:]
                    pending.clear()
                    pending.extend(prev)
                    flush_pending()
                    pending.extend(this_pair)
                flush_pending()

            # --- tail: raw scores PSUM -> SBUF -> HBM (softmax on host).
            # out DMAs ride sync + scalar; the gpsimd queue drains slowly
            # at teardown so keep it clean at the end ---
            with tc.tile_pool(name="epi", bufs=1) as epool:
                scores_sb = epool.tile([U, FD], f32, name="scores_sb")
                H = FD // 2
                nc.vector.tensor_copy(scores_sb[:, 0:H], scU[:, 0:H])
                nc.sync.dma_start(out[:, 0:H], scores_sb[:, 0:H])
                nc.scalar.activation(
                    scores_sb[:, H:FD], scU[:, H:FD], func=AF.Copy
                )
                nc.scalar.dma_start(out[:, H:FD], scores_sb[:, H:FD])

    nc.compile()
    return nc


def _assign(lengths):
    """Bin-pack batches (weight = #units) onto 8 cores, 8 batches each.

    Returns (per_core_batches, U) where per_core_batches[i] is a list of
    8 global batch indices and U is the max unit count.
    """
    w = [max(1, math.ceil(l / CH)) for l in lengths]
    order = sorted(range(B), key=lambda b: -w[b])
    loads = [0] * N_CORES
    members = [[] for _ in range(N_CORES)]
    for b in order:
        cands = [i for i in range(N_CORES) if len(members[i]) < BPC]
        i = min(cands, key=lambda i: (loads[i], len(members[i])))
        members[i].append(b)
        loads[i] += w[b]
    U = max(loads)
    return members, U


def _prep_inputs(decoder_state, encoder_outputs, input_mask, W_transform,
                 b_transform, v_scorer, members, P):
    bf16 = ml_dtypes.bfloat16
    f8 = ml_dtypes.float8_e4m3
    U = GRP * P
    W_dec = W_transform[:D]
    W_enc = W_transform[D:]

    # fp8 half of W (E-rows 0..EF), x64 scale, pr-major for DoubleRow:
    # contraction row e = pr*256 + ko*128 + partition
    w8 = (W_enc[:EF] * W_SCALE).astype(f8)  # [EF, D]
    wenc8_h = np.ascontiguousarray(
        w8.reshape(NPASS, 2, 128, ND, 128).transpose(2, 0, 1, 3, 4)
    )  # [128, NPASS, 2, ND, 128]
    wb = (W_enc[EF:] * W_SCALE).astype(bf16)  # [E-EF, D]
    wencb_h = np.ascontiguousarray(
        wb.reshape(NEB, 128, ND, 128).transpose(1, 0, 2, 3)
    )  # [128, NEB, ND, 128]
    v_tiles = v_scorer.astype(np.float32).reshape(ND, 128).T.astype(bf16)  # [128, ND]
    vsm_h = np.ascontiguousarray(
        np.repeat(v_tiles[:, :, None], GRP, axis=2)
    )  # [128, ND, GRP]

    # dec_proj on host (0.02% of the FLOPs): [B, D]
    dec_proj = (decoder_state.astype(np.float32) @ W_dec.astype(np.float32)
                + b_transform.astype(np.float32))

    lengths = input_mask.sum(axis=1).astype(int)
    enc8_full = (encoder_outputs[:, :, :EF] * ENC_SCALE).astype(f8)  # [B, S, EF]
    encb_full = (encoder_outputs[:, :, EF:] * ENC_SCALE).astype(bf16)  # [B, S, E-EF]

    in_maps = []
    unit_maps = []  # per core: list of (global_batch, chunk) per unit
    for core in range(N_CORES):
        batches = members[core]
        units = []
        for gb in batches:
            for c in range(max(1, math.ceil(lengths[gb] / CH))):
                units.append((gb, c))
        assert len(units) <= U
        enc8_h = np.zeros((P, 128, NPASS, 2, FD), dtype=f8)
        encb_h = np.zeros((P, 128, NEB, FD), dtype=bf16)
        dproj_h = np.zeros((128, ND, U), dtype=np.float32)  # split into 0:4 / 4:U
        for u, (gb, c) in enumerate(units):
            s0 = c * CH
            s1 = min(s0 + CH, int(lengths[gb]))
            n = s1 - s0
            p, h = divmod(u, GRP)
            col = h * CH
            # [n, EF] -> [EF, n] -> [NPASS, 2, 128, n] -> [128, NPASS, 2, n]
            blk8 = enc8_full[gb, s0:s1].T.reshape(NPASS, 2, 128, n)
            enc8_h[p, :, :, :, col : col + n] = blk8.transpose(2, 0, 1, 3)
            blkb = encb_full[gb, s0:s1].T.reshape(NEB, 128, n)
            encb_h[p, :, :, col : col + n] = blkb.transpose(1, 0, 2)
            dproj_h[:, :, u] = dec_proj[gb].reshape(ND, 128).T
        in_maps.append(
            {
                "enc8": enc8_h,
                "encb": encb_h,
                "wenc8": wenc8_h,
                "wencb": wencb_h,
                "dproj0": np.ascontiguousarray(dproj_h[:, :, 0:GRP]),
                "dprojr": np.ascontiguousarray(dproj_h[:, :, GRP:U]),
                "vsm": vsm_h,
            }
        )
        unit_maps.append(units)
    return in_maps, unit_maps


def kernel(decoder_state, encoder_outputs, input_mask, W_transform, b_transform,
           v_scorer, _trace=False):
    from concourse.bass_utils import run_bass_kernel_spmd

    decoder_state = np.asarray(decoder_state)
    encoder_outputs = np.asarray(encoder_outputs)
    input_mask = np.asarray(input_mask)
    W_transform = np.asarray(W_transform)
    b_transform = np.asarray(b_transform)
    v_scorer = np.asarray(v_scorer)

    lengths = input_mask.sum(axis=1).astype(int)
    members, U = _assign(lengths)
    P = (U + GRP - 1) // GRP

    key = ("nc", P)
    if key not in _cache:
        _cache[key] = _build(P)
    nc = _cache[key]

    in_maps, unit_maps = _prep_inputs(
        decoder_state, encoder_outputs, input_mask, W_transform, b_transform,
        v_scorer, members, P
    )
    res = run_bass_kernel_spmd(nc, in_maps, core_ids=list(range(N_CORES)), trace=_trace)

    # gather raw scores, then masked softmax on host (f64 exp, f32 out)
    scores = np.full((B, S), NEG_INF, dtype=np.float32)
    for core in range(N_CORES):
        o = res.results[core]["out"]  # [U, FD]
        for u, (gb, c) in enumerate(unit_maps[core]):
            h = u % GRP
            n = min(CH, int(lengths[gb]) - c * CH)
            scores[gb, c * CH : c * CH + n] = o[u, h * CH : h * CH + n]
    smax = scores.max(axis=1, keepdims=True)
    ex = np.exp((scores - smax).astype(np.float64))
    out_full = (ex / ex.sum(axis=1, keepdims=True)).astype(np.float32)
    if _trace:
        _cache["last_result"] = res
    return out_full


# revision 27
# speedup vs baseline: 1.0677x; 1.0071x over previous
"""Bahdanau-style attention kernel for Trainium2, SPMD across 8 NeuronCores.

Math (per batch row b):
    dec_proj = decoder_state @ W_dec + b_transform            # [D]
    enc_proj = encoder_outputs[b] @ W_enc                     # [S, D]
    feats    = tanh(enc_proj + dec_proj)                      # [S, D]
    scores   = feats @ v_scorer                               # [S]
    probs    = softmax(where(mask, scores, -1e9))             # [S]

Distribution: data-parallel on batch (8 batches per core, weights
replicated), with mask-aware work packing: the mask is length-style, so
positions >= length contribute exactly 0 to the output. Work is split
into units of (batch, 128-column s-chunk) covering only unmasked
columns, and units are bin-packed across the 8 cores. Units are fused
into QUADS (4 x 128 cols) so the enc matmuls run at free-dim 512 (a
full PSUM bank). The Bass graph is parametrized only by P (quads per
core), so graphs are cached per P.

The device computes raw scores only (99.97% of the FLOPs); dec_proj
(0.02%) and the final masked softmax (tiny) run on host in f32/f64 --
this removes the whole device epilogue (mask matmul, exp, bf16 sums,
reciprocal, rescale) and shortens the tail.

Precision: the enc_proj contraction (E=1024) is split 768/256:
  - E-rows 0-767 run as fp8 e4m3 DoubleRow matmuls (2 rows/PE-cell,
    2x bf16 FLOP rate). enc is pre-scaled x16 and W_enc x64 to dodge
    fp8 subnormals; the 2^10 product scale is undone downstream.
  - E-rows 768-1023 stay bf16 (same x16/x64 scaling so both parts
    share one PSUM accumulation group).
  Measured end-to-end rel err ~1.84e-2 vs the f32 reference, under the
  2e-2 gate; fp8 on the full contraction measures ~2.2e-2 in numpy
  simulation (worse on HW) and is not viable.

Device-side formulation (per core, P quads = U units):
  - enc_projT[d, s] per quad in one [128, 512] PSUM bank: 3 DoubleRow
    matmuls (256 contraction rows each) + 2 bf16 matmuls (128 rows).
  - the PSUM drain is split across engines: DVE adds the per-unit
    dec_proj bias (x1024 pre-scaled) for quarters 0-1 then ScalarE
    applies a no-bias tanh to them, and ScalarE handles quarters 2-3
    directly as biased tanh ACTs; output ft [128, 4, 128] bf16.
  - Scores: ONE matmul per (quad, d-tile): lhsT = a [128, U] one-hot-
    quad selector (stride trick: 4 consecutive v-copies sliced per
    quad; PE out base partition must be 0/32/64 so partition-sliced
    outs are not an option) x rhs = ft [128, 512] -> accumulates rows
    4p..4p+3 of scU [U, 512], zeros elsewhere. Row u only has valid
    scores in its own 128-col chunk (h = u%4); the junk in other
    chunks is discarded on host. This quarters the score-matmul
    instruction count vs one-hot per-unit matmuls (same streamed
    cycles, far less per-instruction overhead).
  - Score matmuls are delayed by one quad so the in-order PE never
    stalls on the DVE+ACT drain pipeline.

Startup: the first DR matmul only needs 128KB of weights + 128KB of
rhs, so wenc8 / quad-0 enc8 / quad-0 encb are DMA'd in pass-sized
pieces, spread over four rings (scalar/sync/gpsimd/vector), and quad 0
runs its matmuls pr-major (pass-major) with paired bf16 closes so
compute starts as soon as the first pieces land. Steady-state quads
stay t-major (drains spread evenly) with whole-tile DMAs on the
sync (fp8) and gpsimd (bf16) rings.
"""

import math

import numpy as np
import ml_dtypes

B, S, E, D = 64, 1024, 1024, 512
N_CORES = 8
BPC = B // N_CORES  # batches per core
ND = D // 128  # 4 d-tiles
CH = 128  # columns per work unit
GRP = 4  # units fused into one matmul tile
FD = GRP * CH  # matmul free dim (512, a full PSUM bank)
EF = 768  # contraction rows done in fp8 (0..767)
NPASS = EF // 256  # 3 DoubleRow passes (256 rows each)
NEB = (E - EF) // 128  # 2 bf16 e-tiles (rows 768..1023)
ENC_SCALE = 16.0
W_SCALE = 64.0
PSUM_SCALE = 1.0 / (ENC_SCALE * W_SCALE)
NEG_INF = -1000000000.0

_cache = {}


def _build(P):
    """Build + compile the SPMD graph for P unit-quads per core."""
    from contextlib import ExitStack

    import concourse.bass as bass
    import concourse.tile as tile
    from concourse import bacc, mybir

    f32 = mybir.dt.float32
    bf16 = mybir.dt.bfloat16
    f8 = mybir.dt.float8e4
    AF = mybir.ActivationFunctionType
    DR = mybir.MatmulPerfMode.DoubleRow

    U = GRP * P

    nc = bacc.Bacc(
        "TRN2", target_bir_lowering=False, debug=False, num_devices=N_CORES
    )

    # pr-major weight layouts so quad-0's pass-piece DMAs are contiguous
    enc8 = nc.dram_tensor("enc8", [P, 128, NPASS, 2, FD], f8, kind="ExternalInput").ap()
    encb = nc.dram_tensor("encb", [P, 128, NEB, FD], bf16, kind="ExternalInput").ap()
    wenc8 = nc.dram_tensor("wenc8", [128, NPASS, 2, ND, 128], f8, kind="ExternalInput").ap()
    wencb = nc.dram_tensor("wencb", [128, NEB, ND, 128], bf16, kind="ExternalInput").ap()
    # quad-0's dproj slice is its own (contiguous) tensor so its DMA is a
    # handful of big packets, not 512 16-byte strided ones
    dproj0 = nc.dram_tensor("dproj0", [128, ND, GRP], f32, kind="ExternalInput").ap()
    dprojr = nc.dram_tensor("dprojr", [128, ND, U - GRP], f32, kind="ExternalInput").ap()
    # only the 4 nonzero v-columns come over DMA; the [128, ND, 2U] one-hot-
    # quad selector (stride trick: slice [:, t, U-4p : 2U-4p] is a [128, U]
    # matrix whose only nonzero columns are 4p..4p+3, each = v tile t) is
    # assembled on device by the otherwise-idle DVE to save startup HBM BW
    vsm = nc.dram_tensor("vsm", [128, ND, GRP], bf16, kind="ExternalInput").ap()
    out = nc.dram_tensor("out", [U, FD], f32, kind="ExternalOutput").ap()

    with tile.TileContext(nc) as tc:
        with ExitStack() as ctx:
            const = ctx.enter_context(tc.tile_pool(name="const", bufs=1))
            e8_pool = ctx.enter_context(tc.tile_pool(name="e8p", bufs=5))
            eb_pool = ctx.enter_context(tc.tile_pool(name="ebp", bufs=5))
            fpool = ctx.enter_context(tc.tile_pool(name="feats", bufs=10))
            fpre_pool = ctx.enter_context(tc.tile_pool(name="fpre", bufs=4))

            # PE warmup: the memset is the very first gpsimd instruction so
            # the dependency-free dummy matmuls can start ASAP, fill the
            # startup DMA wait, and trip the HAM clock-gate to 2.4 GHz.
            # Small tile -> short memset -> warmups start right after the
            # fixed ~6us engine preamble.
            warm_sb = const.tile([128, 384], bf16)
            nc.gpsimd.memset(warm_sb[:], 0.0)

            # Startup DMA schedule. Dependency tracking is per-TILE, so
            # quad-0's data is split into piece TILES so the first matmuls
            # unblock after 128KB. Ring assignment stays simple (mixing enc
            # pieces onto the scalar queue measures ~3x slower than keeping
            # it weights-only):
            #   scalar: wenc8 pass tiles, quad-0 dproj, dproj rest, v cols
            #   sync:   quad-0 enc8 pass tiles, then whole fp8 quads
            #   gpsimd: wencb/encb quad-0 pieces interleaved, whole bf16 quads
            # NOTE: untagged same-size tiles in a pool rotate through one
            # slot ring -- every piece tile needs its own tag (bufs=1)
            wenc8_prs = []
            for pr in range(NPASS):
                t8 = const.tile([128, 2, ND, 128], f8, tag=f"w8p{pr}", name="w8p")
                nc.scalar.dma_start(t8[:], wenc8[:, pr])
                wenc8_prs.append(t8)
            e80_prs = []
            for pr in range(NPASS):
                tp = e8_pool.tile(
                    [128, 2, FD], f8, tag=f"e80p{pr}", name="e80p", bufs=1
                )
                nc.sync.dma_start(tp[:], enc8[0][:, pr])
                e80_prs.append(tp)
            dproj0_sb = const.tile([128, ND, GRP], f32)
            nc.scalar.dma_start(dproj0_sb[:], dproj0)
            # wencb rides sync right after quad-0's fp8 (the gpsimd queue
            # only sustains ~52 B/ns and was gating quad-0's bf16 close)
            wencb_sb = const.tile([128, NEB, ND, 128], bf16)
            eb0_es = []
            for e in range(NEB):
                nc.sync.dma_start(wencb_sb[:, e], wencb[:, e])
                te_ = eb_pool.tile(
                    [128, FD], bf16, tag=f"eb0e{e}", name="eb0e", bufs=1
                )
                nc.gpsimd.dma_start(te_[:], encb[0][:, e])
                eb0_es.append(te_)
            e8s, ebs = [None], [None]
            npre = min(3, P)
            for p in range(1, npre):
                e8t = e8_pool.tile([128, NPASS, 2, FD], f8, tag="e8", name="e8")
                nc.sync.dma_start(e8t[:], enc8[p])
                e8s.append(e8t)
                ebt = eb_pool.tile([128, NEB, FD], bf16, tag="eb", name="eb")
                nc.gpsimd.dma_start(ebt[:], encb[p])
                ebs.append(ebt)
            dprojr_sb = const.tile([128, ND, U - GRP], f32)
            nc.scalar.dma_start(dprojr_sb[:], dprojr)
            vsm_sb = const.tile([128, ND, GRP], bf16, tag="vsm", name="vsm")
            nc.scalar.dma_start(vsm_sb[:], vsm)
            # DVE assembles the one-hot-quad selector from the v columns
            vQ_sb = const.tile([128, ND, 2 * U], bf16)
            nc.vector.memset(vQ_sb[:], 0.0)
            nc.vector.tensor_copy(vQ_sb[:, :, U : U + GRP], vsm_sb[:])

            with tc.tile_pool(name="warmp", bufs=1, space="PSUM") as wpool:
                wps = wpool.tile([128, 256], f32, name="wps")
                for _ in range(10):
                    nc.tensor.matmul(
                        wps[:],
                        lhsT=warm_sb[:, 0:128],
                        rhs=warm_sb[:, 128:384],
                        start=True,
                        stop=True,
                        skip_group_check=True,
                    )

            spsum = ctx.enter_context(tc.tile_pool(name="spsum", bufs=1, space="PSUM"))
            scU = spsum.tile([U, FD], f32, name="scU")
            n_sc_mms = P * ND
            state = {"count": 0}
            pending = []  # delayed score MMs: (p, t, ft)

            def flush_pending():
                for (p, t, ft) in pending:
                    state["count"] += 1
                    nc.tensor.matmul(
                        scU[:],
                        lhsT=vQ_sb[:, t, U - GRP * p : 2 * U - GRP * p],
                        rhs=ft[:],
                        start=(state["count"] == 1),
                        stop=(state["count"] == n_sc_mms),
                        skip_group_check=True,
                    )
                pending.clear()

            def drain(ps, p, t):
                """PSUM [128, 512] -> ft bf16 via DVE scale+bias + ScalarE tanh.

                DVE takes 3 of the 4 quarters so ScalarE (whose tanh stream
                is the irreducible part) runs just 2 instructions per d-tile.
                """
                NV = 3
                dp = dproj0_sb if p == 0 else dprojr_sb
                u0 = 0 if p == 0 else GRP * (p - 1)
                fpre = fpre_pool.tile([128, NV, CH], bf16, tag="fp", name="fp")
                for h in range(NV):
                    # fpre = ps * 2^-10 + dproj[u] in one DVE op (scaled
                    # imm first, then per-partition bias add)
                    nc.vector.tensor_scalar(
                        fpre[:, h, :],
                        ps[:, h, :],
                        PSUM_SCALE,
                        dp[:, t, u0 + h : u0 + h + 1],
                        mybir.AluOpType.mult,
                        mybir.AluOpType.add,
                    )
                ft = fpool.tile([128, GRP, CH], bf16, tag="ft", name="ft")
                for h in range(NV, GRP):
                    nc.scalar.activation(
                        ft[:, h, :],
                        ps[:, h, :],
                        func=AF.Tanh,
                        bias=dp[:, t, u0 + h : u0 + h + 1],
                        scale=PSUM_SCALE,
                    )
                nc.scalar.activation(
                    ft[:, 0:NV, :], fpre[:], func=AF.Tanh, scale=1.0
                )
                pending.append((p, t, ft))

            # --- main loop over quads ---
            with tc.tile_pool(name="mpsum", bufs=7, space="PSUM") as mpsum:
                for p in range(P):
                    if p == 0:
                        e8t, ebt = None, None
                    elif p < npre:
                        e8t, ebt = e8s[p], ebs[p]
                    else:
                        e8t = e8_pool.tile([128, NPASS, 2, FD], f8, tag="e8", name="e8")
                        nc.sync.dma_start(e8t[:], enc8[p])
                        ebt = eb_pool.tile([128, NEB, FD], bf16, tag="eb", name="eb")
                        # bf16 tile rides the gpsimd ring: one ring tops out
                        # at ~102 B/ns; splitting fp8/bf16 across two rings
                        # keeps DMA under the PE floor
                        nc.gpsimd.dma_start(ebt[:], encb[p])
                    prev = list(pending)
                    pending.clear()
                    pss = [
                        mpsum.tile([128, GRP, CH], f32, tag="mp", name="mp")
                        for _ in range(ND)
                    ]
                    if p == 0:
                        # quad 0: pass-major so compute starts when the first
                        # 128KB weight + rhs piece tiles land; bf16 closes
                        # paired per d-tile so drains start at matmul 14
                        for pr in range(NPASS):
                            for t in range(ND):
                                nc.tensor.matmul(
                                    pss[t][:],
                                    lhsT=wenc8_prs[pr][:, :, t, :],
                                    rhs=e80_prs[pr][:],
                                    start=(pr == 0),
                                    stop=False,
                                    perf_mode=DR,
                                )
                        for t in range(ND):
                            for e in range(NEB):
                                nc.tensor.matmul(
                                    pss[t][:],
                                    lhsT=wencb_sb[:, e, t, :],
                                    rhs=eb0_es[e][:],
                                    start=False,
                                    stop=(e == NEB - 1),
                                )
                            drain(pss[t], p, t)
                    else:
                        # steady state: t-major, fp8 first (their data leads
                        # the DMA stream), bf16 closes each group, drain per
                        # d-tile so ScalarE/DVE work is spread evenly
                        for t in range(ND):
                            for pr in range(NPASS):
                                nc.tensor.matmul(
                                    pss[t][:],
                                    lhsT=wenc8_prs[pr][:, :, t, :],
                                    rhs=e8t[:, pr, :, :],
                                    start=(pr == 0),
                                    stop=False,
                                    perf_mode=DR,
                                )
                        for t in range(ND):
                            for e in range(NEB):
                                nc.tensor.matmul(
                                    pss[t][:],
                                    lhsT=wencb_sb[:, e, t, :],
                                    rhs=ebt[:, e, :],
                                    start=False,
                                    stop=(e == NEB - 1),
                                )
                            drain(pss[t], p, t)
                    # emit previous quad's score MMs now (their tanh inputs
                    # are ready, so PE doesn't stall on ACT)
                    this_pair = pending[:]
                    pending.clear()
                    pending.extend(prev)
                    flush_pending()
                    pending.extend(this_pair)
                flush_pending()

            # --- tail: raw scores PSUM -> SBUF -> HBM (softmax on host).
            # out DMAs ride sync + scalar; the gpsimd queue drains slowly
            # at teardown so keep it clean at the end ---
            with tc.tile_pool(name="epi", bufs=1) as epool:
                scores_sb = epool.tile([U, FD], f32, name="scores_sb")
                H = FD // 2
                nc.vector.tensor_copy(scores_sb[:, 0:H], scU[:, 0:H])
                nc.sync.dma_start(out[:, 0:H], scores_sb[:, 0:H])
                nc.scalar.activation(
                    scores_sb[:, H:FD], scU[:, H:FD], func=AF.Copy
                )
                nc.scalar.dma_start(out[:, H:FD], scores_sb[:, H:FD])

    nc.compile()
    return nc


def _assign(lengths):
    """Bin-pack batches (weight = #units) onto 8 cores, 8 batches each.

    Returns (per_core_batches, U) where per_core_batches[i] is a list of
    8 global batch indices and U is the max unit count.
    """
    w = [max(1, math.ceil(l / CH)) for l in lengths]
    order = sorted(range(B), key=lambda b: -w[b])
    loads = [0] * N_CORES
    members = [[] for _ in range(N_CORES)]
    for b in order:
        cands = [i for i in range(N_CORES) if len(members[i]) < BPC]
        i = min(cands, key=lambda i: (loads[i], len(members[i])))
        members[i].append(b)
        loads[i] += w[b]
    U = max(loads)
    return members, U


def _prep_inputs(decoder_state, encoder_outputs, input_mask, W_transform,
                 b_transform, v_scorer, members, P):
    bf16 = ml_dtypes.bfloat16
    f8 = ml_dtypes.float8_e4m3
    U = GRP * P
    W_dec = W_transform[:D]
    W_enc = W_transform[D:]

    # fp8 half of W (E-rows 0..EF), x64 scale, pr-major for DoubleRow:
    # contraction row e = pr*256 + ko*128 + partition
    w8 = (W_enc[:EF] * W_SCALE).astype(f8)  # [EF, D]
    wenc8_h = np.ascontiguousarray(
        w8.reshape(NPASS, 2, 128, ND, 128).transpose(2, 0, 1, 3, 4)
    )  # [128, NPASS, 2, ND, 128]
    wb = (W_enc[EF:] * W_SCALE).astype(bf16)  # [E-EF, D]
    wencb_h = np.ascontiguousarray(
        wb.reshape(NEB, 128, ND, 128).transpose(1, 0, 2, 3)
    )  # [128, NEB, ND, 128]
    v_tiles = v_scorer.astype(np.float32).reshape(ND, 128).T.astype(bf16)  # [128, ND]
    vsm_h = np.ascontiguousarray(
        np.repeat(v_tiles[:, :, None], GRP, axis=2)
    )  # [128, ND, GRP]

    # dec_proj on host (0.02% of the FLOPs): [B, D]
    dec_proj = (decoder_state.astype(np.float32) @ W_dec.astype(np.float32)
                + b_transform.astype(np.float32))

    lengths = input_mask.sum(axis=1).astype(int)
    enc8_full = (encoder_outputs[:, :, :EF] * ENC_SCALE).astype(f8)  # [B, S, EF]
    encb_full = (encoder_outputs[:, :, EF:] * ENC_SCALE).astype(bf16)  # [B, S, E-EF]

    in_maps = []
    unit_maps = []  # per core: list of (global_batch, chunk) per unit
    for core in range(N_CORES):
        batches = members[core]
        units = []
        for gb in batches:
            for c in range(max(1, math.ceil(lengths[gb] / CH))):
                units.append((gb, c))
        assert len(units) <= U
        enc8_h = np.zeros((P, 128, NPASS, 2, FD), dtype=f8)
        encb_h = np.zeros((P, 128, NEB, FD), dtype=bf16)
        dproj_h = np.zeros((128, ND, U), dtype=np.float32)  # split into 0:4 / 4:U
        for u, (gb, c) in enumerate(units):
            s0 = c * CH
            s1 = min(s0 + CH, int(lengths[gb]))
            n = s1 - s0
            p, h = divmod(u, GRP)
            col = h * CH
            # [n, EF] -> [EF, n] -> [NPASS, 2, 128, n] -> [128, NPASS, 2, n]
            blk8 = enc8_full[gb, s0:s1].T.reshape(NPASS, 2, 128, n)
            enc8_h[p, :, :, :, col : col + n] = blk8.transpose(2, 0, 1, 3)
            blkb = encb_full[gb, s0:s1].T.reshape(NEB, 128, n)
            encb_h[p, :, :, col : col + n] = blkb.transpose(1, 0, 2)
            dproj_h[:, :, u] = dec_proj[gb].reshape(ND, 128).T
        in_maps.append(
            {
                "enc8": enc8_h,
                "encb": encb_h,
                "wenc8": wenc8_h,
                "wencb": wencb_h,
                "dproj0": np.ascontiguousarray(dproj_h[:, :, 0:GRP]),
                "dprojr": np.ascontiguousarray(dproj_h[:, :, GRP:U]),
                "vsm": vsm_h,
            }
        )
        unit_maps.append(units)
    return in_maps, unit_maps


def kernel(decoder_state, encoder_outputs, input_mask, W_transform, b_transform,
           v_scorer, _trace=False):
    from concourse.bass_utils import run_bass_kernel_spmd

    decoder_state = np.asarray(decoder_state)
    encoder_outputs = np.asarray(encoder_outputs)
    input_mask = np.asarray(input_mask)
    W_transform = np.asarray(W_transform)
    b_transform = np.asarray(b_transform)
    v_scorer = np.asarray(v_scorer)

    lengths = input_mask.sum(axis=1).astype(int)
    members, U = _assign(lengths)
    P = (U + GRP - 1) // GRP

    key = ("nc", P)
    if key not in _cache:
        _cache[key] = _build(P)
    nc = _cache[key]

    in_maps, unit_maps = _prep_inputs(
        decoder_state, encoder_outputs, input_mask, W_transform, b_transform,
        v_scorer, members, P
    )
    res = run_bass_kernel_spmd(nc, in_maps, core_ids=list(range(N_CORES)), trace=_trace)

    # gather raw scores, then masked softmax on host (f64 exp, f32 out)
    scores = np.full((B, S), NEG_INF, dtype=np.float32)
    for core in range(N_CORES):
        o = res.results[core]["out"]  # [U, FD]
        for u, (gb, c) in enumerate(unit_maps[core]):
            h = u % GRP
            n = min(CH, int(lengths[gb]) - c * CH)
            scores[gb, c * CH : c * CH + n] = o[u, h * CH : h * CH + n]
    smax = scores.max(axis=1, keepdims=True)
    ex = np.exp((scores - smax).astype(np.float64))
    out_full = (ex / ex.sum(axis=1, keepdims=True)).astype(np.float32)
    if _trace:
        _cache["last_result"] = res
    return out_full
